# revision 15
# baseline (speedup 1.0000x reference)
"""Trainium2 Bass kernel for AdaptivePrototypeContrastiveLoss.

Strategy
--------
Host (cheap, O(N*D) bookkeeping):
  * closed-form momentum EMA + LAPACK QR -> new prototypes  [7,256]
  * row-normalize feats, stable-sort rows by label, append 7 per-class
    sum columns (Y) so the tiny "positive" matmul rides the main sweep
  * precompute per-row constants (alpha/beta/valid/onehot)

Device (8 NeuronCores, SPMD, no collectives; all O(N^2) work):
  * row-shard: each core owns 8 row-tiles of 128 rows (64 tiles cover
    rows 0..8191); the last 7 rows' column sweep (row-tile 64) is
    split column-wise across all 8 cores as class-pure 512-chunks
  * per row-tile: G = rows @ feats^T via PE (bf16 operands, f32 PSUM,
    K=256, 512-col chunks grouped into 1536-col PSUM supertiles)
  * ACT computes exp(A*sim + BIAS) from PSUM into bf16 scratch; DVE
    reduces the class sub-ranges (columns are label-sorted so class
    segments are contiguous and identical on all cores -> the graph
    stays SPMD-uniform)
  * neg_i = total - own-class (selected via shipped onehot); the
    global max subtraction is replaced by the constant M0=12.5 (the
    max only enters through ~1e-8-scale eps terms, verified offline)
  * per-core output: 128-partition partial sums of thresholded loss
    + 3 column-chunk exp sums for the shared row-tile 64
Host: combine 8x[128,8] partials -> scalar.
"""

import ml_dtypes
import numpy as np

import concourse.bass as bass
import concourse.tile as tile
from concourse import mybir
from concourse.bass_utils import run_bass_kernel_spmd

# ---- problem constants (hardcoded per spec) ----
TEMP = 0.08
EPS = 1e-8
GAMMA = 0.99
BETA = 0.5 * (1.0 - GAMMA)
B, D, C = 8192, 256, 7
N = B + C                      # 8199 rows/cols of the score matrix
NCORES = 8
NT = 8                         # full row-tiles per core (8*8*128 = 8192)
ROWS_PER_CORE = NT * 128       # 1024
NPAD = 8704                    # columns padded to 17*512
NF = NPAD + 16                 # + 7 Y columns + zero cols (16-aligned)
SUPER = 2048                   # psum supertile width (4 banks)
T8W = 1536                     # per-core share of row-tile 64's columns
M0 = 12.5                      # constant stand-in for the global max
A_SCALE = 0.5 / float(np.float32(TEMP))
BIAS = (0.5 + EPS) / float(np.float32(TEMP)) - M0

F32 = mybir.dt.float32
BF16 = mybir.dt.bfloat16
FP8 = mybir.dt.float8e4
FP8NP = mybir.dt.np(mybir.dt.float8e4)
ALU = mybir.AluOpType
ACTF = mybir.ActivationFunctionType


def _split_multi_waits(nc):
    """This container's walrus accepts only ONE sync wait per instruction;
    split extra waits into standalone single-wait EventSemaphore insts."""
    n_new = 0
    for func in nc.m.functions:
        for blk in func.blocks:
            new_insts = []
            for inst in blk.instructions:
                si = getattr(inst, "sync_info", None)
                waits = list(si.on_wait) if si and si.on_wait else []
                if len(waits) > 1:
                    for i, w in enumerate(waits[:-1]):
                        n_new += 1
                        ev = mybir.InstEventSemaphore(
                            name=f"{inst.name}-wsplit{i}",
                            engine=inst.engine,
                            ins=[],
                            outs=[],
                            sync_info=mybir.SyncInfo(on_wait=[w], on_update=[]),
                            bass_nofuse=True,
                        )
                        new_insts.append(ev)
                    si.on_wait = [waits[-1]]
                new_insts.append(inst)
            blk.instructions = new_insts
    return n_new


def _host_prep(features, labels, prototypes, momentums):
    # (subranges computed below before per-core metadata uses it)
    features = np.asarray(features, dtype=np.float32)
    labels = np.asarray(labels).astype(np.int64)
    prototypes = np.asarray(prototypes, dtype=np.float32)
    momentums = np.asarray(momentums, dtype=np.float32)

    # ---- prototype update: closed form of the sequential EMA scan ----
    counts_feat = np.bincount(labels, minlength=C)
    rank = np.zeros(B, dtype=np.int64)
    seen = np.zeros(C, dtype=np.int64)
    for i, l in enumerate(labels):
        rank[i] = seen[l]
        seen[l] += 1
    w = BETA * (GAMMA ** (counts_feat[labels] - 1 - rank).astype(np.float64))
    S = np.zeros((C, B))
    S[labels, np.arange(B)] = w
    m_final = S @ features.astype(np.float64)
    wsum = np.bincount(labels, weights=w, minlength=C)
    m_final -= wsum[:, None] * prototypes.astype(np.float64)
    m_final += (GAMMA ** counts_feat.astype(np.float64))[:, None] * momentums.astype(
        np.float64
    )
    target = prototypes.astype(np.float64) + m_final
    q, _ = np.linalg.qr(target.T.astype(np.float32))
    new_protos = q.T.astype(np.float32)

    # ---- normalized, label-sorted gram operands ----
    feats = np.concatenate([features, new_protos], 0)
    labs = np.concatenate([labels, np.arange(C, dtype=np.int64)])
    nrm = np.linalg.norm(feats.astype(np.float64), axis=-1)
    fhat = feats.astype(np.float64) / nrm[:, None]
    perm = np.argsort(labs, kind="stable")
    fs = fhat[perm]
    ls = labs[perm]
    counts_all = np.bincount(ls, minlength=C)          # includes protos
    bounds = np.concatenate([[0], np.cumsum(counts_all)])  # class col ranges

    fs32 = fs.astype(np.float32)
    Y = np.zeros((D, 8), dtype=np.float64)
    for c in range(C):
        Y[:, c] = fs[bounds[c]:bounds[c + 1]].sum(0)

    ftpad = np.zeros((NF, D), dtype=np.float32)
    ftpad[:N] = fs32
    ftpad[NPAD:NPAD + 8] = Y.T.astype(np.float32)
    ft = np.ascontiguousarray(ftpad.T.reshape(2, 128, NF)).astype(
        ml_dtypes.bfloat16
    )  # [k-half, partition, col]

    # ---- ACT sub-ranges: class segments x supertile edges (global) ----
    super_edges = list(range(0, NPAD, SUPER)) + [N]
    edges = sorted(set([int(b) for b in bounds] + super_edges))
    edges = [e for e in edges if e <= N]
    subranges = []  # (super_idx, off_in_super, length, class_id)
    for a, b in zip(edges[:-1], edges[1:]):
        if a >= N:
            break
        cls = int(np.searchsorted(bounds, a, side="right") - 1)
        sup = a // SUPER
        subranges.append((sup, a - sup * SUPER, b - a, cls))
    slot_ranges = []  # per-class slot ranges (contiguous in list order)
    for c in range(C):
        idxs = [i for i, sr in enumerate(subranges) if sr[3] == c]
        slot_ranges.append((min(idxs), max(idxs) + 1))
    n_slots = len(subranges)

    # ---- per-row constants ----
    cnt = counts_all[ls] - 1
    selfsim = (fs32.astype(np.float64) ** 2).sum(1)
    inv = 1.0 / (cnt.astype(np.float64) + EPS)
    alpha_all = A_SCALE * inv
    beta_all = (-A_SCALE * selfsim + BIAS * cnt) * inv

    # ---- shared row-tile 64 (last 7 rows), column-split across cores ----
    t8rows = np.ascontiguousarray(
        ftpad[B:B + 128].T.reshape(2, 128, 128)
    ).astype(ml_dtypes.bfloat16)
    chunk_cols = []  # class-pure 512-col chunks (global col indices)
    chunk_cls = []
    for c in range(C):
        cols = np.arange(bounds[c], bounds[c + 1])
        for o in range(0, len(cols), 512):
            chunk_cols.append(cols[o:o + 512])
            chunk_cls.append(c)
    n_cpc = T8W // 512  # chunks per core
    while len(chunk_cols) < NCORES * n_cpc:
        chunk_cols.append(np.zeros(0, dtype=np.int64))
        chunk_cls.append(-1)
    t8meta = []  # (class, n_pad) per chunk for the host-side combine
    ft_np = np.asarray(ft)
    t8cols_per_core = []
    for core in range(NCORES):
        arr = np.zeros((2, 128, T8W), dtype=ml_dtypes.bfloat16)
        for j in range(n_cpc):
            ci = core * n_cpc + j
            cols = chunk_cols[ci]
            arr[:, :, j * 512:j * 512 + len(cols)] = ft_np[:, :, cols]
            t8meta.append((chunk_cls[ci], 512 - len(cols)))
        t8cols_per_core.append(arr)

    per_core = []
    for core in range(NCORES):
        base = core * ROWS_PER_CORE
        rows_kt = np.ascontiguousarray(
            ftpad[base:base + ROWS_PER_CORE].T.reshape(2, 128, ROWS_PER_CORE)
        ).astype(ml_dtypes.bfloat16)

        onehot = np.zeros((NT, 128, 8), dtype=np.float32)
        rowmeta = np.zeros((128, 3 * NT), dtype=np.float32)  # alpha|beta|valid
        for t in range(NT):
            for p in range(128):
                g = base + t * 128 + p
                onehot[t, p, ls[g]] = 1.0
                rowmeta[p, t] = alpha_all[g]
                rowmeta[p, NT + t] = beta_all[g]
                rowmeta[p, 2 * NT + t] = 1.0
        per_core.append(
            {
                "ft": ft,
                "rows": rows_kt,
                "onehot": onehot,
                "rowmeta": rowmeta,
                "t8rows": t8rows,
                "t8cols": t8cols_per_core[core],
            }
        )

    host = {
        "ls": ls, "bounds": bounds, "counts_all": counts_all, "fs": fs,
        "Y": Y, "t8meta": t8meta, "selfsim": selfsim, "cnt": cnt,
    }
    return per_core, subranges, slot_ranges, n_slots, host


def _build_graph(subranges, slot_ranges, n_slots, bounds):
    nc = bass.Bass()
    ft_d = nc.declare_dram_parameter("ft", [2, 128, NF], BF16, isOutput=False)
    rows_d = nc.declare_dram_parameter(
        "rows", [2, 128, ROWS_PER_CORE], BF16, isOutput=False
    )
    oh_d = nc.declare_dram_parameter("onehot", [NT, 128, 8], F32, isOutput=False)
    meta_d = nc.declare_dram_parameter("rowmeta", [128, 3 * NT], F32, isOutput=False)
    t8r_d = nc.declare_dram_parameter("t8rows", [2, 128, 128], BF16, isOutput=False)
    t8c_d = nc.declare_dram_parameter("t8cols", [2, 128, T8W], BF16, isOutput=False)
    out_d = nc.declare_dram_parameter("out", [128, 8], F32, isOutput=True)

    n_super = (NPAD + SUPER - 1) // SUPER  # 6 (last covers Y cols too)
    super_chunks = []  # per supertile: (ft_off, width, psum_off)
    for s in range(n_super):
        lo = s * SUPER
        hi = min(lo + SUPER, NPAD)
        chunks = [(o, 512, o - lo) for o in range(lo, hi, 512)]
        if s == n_super - 1:
            chunks.append((NPAD, 8, hi - lo))  # Y columns
        super_chunks.append(chunks)
    y_psum_off = NPAD - (n_super - 1) * SUPER  # offset of Y cols in last super

    with tile.TileContext(nc) as tc:
        with (
            tc.tile_pool(name="persist", bufs=1) as persist,
            tc.tile_pool(name="ps", bufs=2, space="PSUM") as psA,
            tc.tile_pool(name="scr", bufs=2) as scrp,
            tc.tile_pool(name="scr8", bufs=2) as scr8p,
            tc.tile_pool(name="slots", bufs=2) as slotp,
            tc.tile_pool(name="small", bufs=4) as small,
        ):
            # --- resident inputs ---
            rows_sb = []
            for k in range(2):
                t_ = persist.tile([128, ROWS_PER_CORE], BF16, tag=f"rows{k}")
                nc.sync.dma_start(out=t_[:], in_=rows_d[k])
                rows_sb.append(t_)
            t8r_sb = []
            for k in range(2):
                t_ = persist.tile([128, 128], BF16, tag=f"t8r{k}")
                nc.sync.dma_start(out=t_[:], in_=t8r_d[k])
                t8r_sb.append(t_)
            t8c_sb = []
            for k in range(2):
                t_ = persist.tile([128, T8W], BF16, tag=f"t8c{k}")
                nc.sync.dma_start(out=t_[:], in_=t8c_d[k])
                t8c_sb.append(t_)
            meta_sb = persist.tile([128, 3 * NT], F32, tag="meta")
            nc.sync.dma_start(out=meta_sb[:], in_=meta_d[:])
            oh_sb = persist.tile([128, NT, 8], F32, tag="oh")
            for t in range(NT):
                nc.sync.dma_start(out=oh_sb[:, t, :], in_=oh_d[t])
            ft_sb = []
            for k in range(2):
                t_ = persist.tile([128, NF], BF16, tag=f"ft{k}")
                ft_sb.append(t_)
            for s in range(n_super):
                lo = s * SUPER
                hi = NF if s == n_super - 1 else lo + SUPER
                for k in range(2):
                    nc.sync.dma_start(out=ft_sb[k][:, lo:hi], in_=ft_d[k, :, lo:hi])

            possel9 = persist.tile([128, NT], F32, tag="possel")
            negsum9 = persist.tile([128, NT], F32, tag="negsum")
            bias_exp = persist.tile([128, 1], F32, tag="bias_exp")
            nc.vector.memset(bias_exp[:], float(BIAS))
            bias_ln = persist.tile([128, 1], F32, tag="bias_ln")
            nc.vector.memset(bias_ln[:], float(EPS))
            out_t = persist.tile([128, 8], F32, tag="out")

            # --- main loop over row-tiles ---
            for t in range(NT):
                scr = scrp.tile([128, NPAD], F32, tag="scr")
                clsum = small.tile([128, 8], F32, tag="csum")
                for s in range(n_super):
                    ps = psA.tile([128, SUPER], F32, tag="ps")
                    reps = [0, 0, 1] if t == 0 else [0, 1]
                    for ri, k in enumerate(reps):
                        last = ri == len(reps) - 1
                        for (off, w, poff) in super_chunks[s]:
                            nc.tensor.matmul(
                                ps[:, poff:poff + w],
                                lhsT=rows_sb[k][:, t * 128:(t + 1) * 128],
                                rhs=ft_sb[k][:, off:off + w],
                                start=(ri == 0),
                                stop=last,
                            )
                    lo = s * SUPER
                    hi_valid = min(lo + SUPER, N)
                    if hi_valid > lo:
                        nc.scalar.activation(
                            scr[:, lo:hi_valid],
                            ps[:, 0:hi_valid - lo],
                            ACTF.Exp,
                            bias=bias_exp[:],
                            scale=float(A_SCALE),
                        )
                    if s == n_super - 1:
                        scr7 = small.tile([128, 7], F32, tag="scr7")
                        nc.vector.tensor_tensor(
                            out=scr7[:],
                            in0=ps[:, y_psum_off:y_psum_off + 7],
                            in1=oh_sb[:, t, 0:7],
                            op=ALU.mult,
                        )
                        nc.vector.reduce_sum(
                            possel9[:, t:t + 1], scr7[:], mybir.AxisListType.X
                        )
                # per-class segment sums straight off the big scratch
                for c in range(C):
                    nc.vector.reduce_sum(
                        clsum[:, c:c + 1],
                        scr[:, int(bounds[c]):int(bounds[c + 1])],
                        mybir.AxisListType.X,
                    )
                stot = small.tile([128, 1], F32, tag="stot")
                nc.vector.reduce_sum(stot[:], clsum[:, 0:7], mybir.AxisListType.X)
                scr7b = small.tile([128, 7], F32, tag="scr7b")
                sown = small.tile([128, 1], F32, tag="sown")
                nc.vector.tensor_tensor(
                    out=scr7b[:], in0=clsum[:, 0:7], in1=oh_sb[:, t, 0:7],
                    op=ALU.mult,
                )
                nc.vector.reduce_sum(sown[:], scr7b[:], mybir.AxisListType.X)
                nc.vector.tensor_tensor(
                    out=negsum9[:, t:t + 1], in0=stot[:], in1=sown[:],
                    op=ALU.subtract,
                )

            # --- shared row-tile 64: this core's column slice ---
            ps8 = psA.tile([128, SUPER], F32, tag="ps")
            ps8b = psA.tile([128, SUPER], F32, tag="ps")
            for k in range(2):
                for j in range(T8W // 512):
                    tgt = ps8 if j < 2 else ps8b
                    nc.tensor.matmul(
                        tgt[:, (j % 2) * 512:(j % 2 + 1) * 512],
                        lhsT=t8r_sb[k][:],
                        rhs=t8c_sb[k][:, j * 512:(j + 1) * 512],
                        start=(k == 0),
                        stop=(k == 1),
                    )
            scr8 = scr8p.tile([128, SUPER], F32, tag="scr8")
            scr8b = scr8p.tile([128, SUPER], F32, tag="scr8")
            nc.scalar.activation(
                scr8[:, 0:1024], ps8[:, 0:1024], ACTF.Exp,
                bias=bias_exp[:], scale=float(A_SCALE),
            )
            nc.scalar.activation(
                scr8b[:, 0:512], ps8b[:, 0:512], ACTF.Exp,
                bias=bias_exp[:], scale=float(A_SCALE),
            )
            for j in range(T8W // 512):
                stile = scr8 if j < 2 else scr8b
                nc.vector.reduce_sum(
                    out_t[:, 2 + j:3 + j], stile[:, (j % 2) * 512:(j % 2 + 1) * 512],
                    mybir.AxisListType.X,
                )

            # --- epilogue: loss, threshold, partial sums ---
            alpha9 = meta_sb[:, 0:NT]
            beta9 = meta_sb[:, NT:2 * NT]
            valid9 = meta_sb[:, 2 * NT:3 * NT]
            pos9 = persist.tile([128, NT], F32, tag="pos9")
            nc.vector.tensor_tensor(
                out=pos9[:], in0=possel9[:], in1=alpha9, op=ALU.mult
            )
            nc.vector.tensor_tensor(out=pos9[:], in0=pos9[:], in1=beta9, op=ALU.add)
            neg9 = persist.tile([128, NT], F32, tag="neg9")
            nc.scalar.activation(
                neg9[:], negsum9[:], ACTF.Ln, bias=bias_ln[:], scale=1.0
            )
            loss9 = persist.tile([128, NT], F32, tag="loss9")
            nc.vector.tensor_tensor(
                out=loss9[:], in0=neg9[:], in1=pos9[:], op=ALU.subtract
            )
            gt9 = persist.tile([128, NT], F32, tag="gt9")
            nc.vector.tensor_scalar(
                out=gt9[:], in0=loss9[:], scalar1=0.0, scalar2=None, op0=ALU.is_gt
            )
            nc.vector.tensor_tensor(out=gt9[:], in0=gt9[:], in1=valid9, op=ALU.mult)
            contrib9 = persist.tile([128, NT], F32, tag="contrib9")
            nc.vector.tensor_tensor(
                out=contrib9[:], in0=loss9[:], in1=gt9[:], op=ALU.mult
            )
            nc.vector.reduce_sum(out_t[:, 0:1], contrib9[:], mybir.AxisListType.X)
            nc.vector.reduce_sum(out_t[:, 1:2], gt9[:], mybir.AxisListType.X)
            nc.sync.dma_start(out=out_d[:], in_=out_t[:])
    return nc


def _combine(results, host):
    """Host-side unshard: merge per-core partials + finish row-tile 64."""
    ls = host["ls"]
    fs, Y = host["fs"], host["Y"]
    loss_sum = 0.0
    cnt_sum = 0.0
    for r in results:
        o = np.asarray(r["out"], dtype=np.float64)
        loss_sum += o[:, 0].sum()
        cnt_sum += o[:, 1].sum()

    # row-tile 64: rows 8192..8198 — class sums from per-core chunk sums
    pad_exp = float(
        ml_dtypes.bfloat16(np.exp(np.float32(BIAS)))
    )  # a zero pad column's exp as the device computes it
    n7 = N - B  # 7
    n_cpc = T8W // 512
    classsum = np.zeros((n7, C), dtype=np.float64)
    for core in range(NCORES):
        o = np.asarray(results[core]["out"], dtype=np.float64)
        for j in range(n_cpc):
            cls, n_pad = host["t8meta"][core * n_cpc + j]
            if cls < 0:
                continue
            classsum[:, cls] += o[:n7, 2 + j] - n_pad * pad_exp
    stot = classsum.sum(1)
    rows_ls = ls[B:N]
    sown = classsum[np.arange(n7), rows_ls]
    neg = np.log(stot - sown + EPS)
    pos_sel = np.einsum("id,di->i", fs[B:N], Y[:, rows_ls])
    selfsim = host["selfsim"][B:N]
    cnt = host["cnt"][B:N]
    pos = (A_SCALE * (pos_sel - selfsim) + BIAS * cnt) / (cnt + EPS)
    loss64 = -pos + neg
    m = loss64 > 0
    loss_sum += loss64[m].sum()
    cnt_sum += m.sum()

    val = loss_sum / max(cnt_sum, 1.0) if cnt_sum > 0 else 0.0
    return np.float32(val)


def _run(features, labels, prototypes, momentums, trace=False, trace_kwargs=None):
    per_core, subranges, slot_ranges, n_slots, host = _host_prep(
        features, labels, prototypes, momentums
    )
    nc = _build_graph(subranges, slot_ranges, n_slots, host["bounds"])
    _split_multi_waits(nc)
    in_maps = [per_core[i] for i in range(NCORES)]
    kw = {}
    if trace:
        kw = dict(trace=True, trace_cores=list(range(NCORES)))
        if trace_kwargs:
            kw["trace_kwargs"] = trace_kwargs
    res = run_bass_kernel_spmd(nc, in_maps, core_ids=list(range(NCORES)), **kw)
    return _combine(res.results, host), res


def kernel(features, labels, prototypes, momentums):
    val, _ = _run(features, labels, prototypes, momentums)
    return np.array(val, dtype=np.float32)


# revision 17
# speedup vs baseline: 1.0686x; 1.0686x over previous
"""Trainium2 Bass kernel for AdaptivePrototypeContrastiveLoss.

Strategy
--------
Host (cheap, O(N*D) bookkeeping):
  * closed-form momentum EMA + LAPACK QR -> new prototypes  [7,256]
  * row-normalize feats, stable-sort rows by label, append 7 per-class
    sum columns (Y) so the tiny "positive" matmul rides the main sweep
  * precompute per-row constants (alpha/beta/valid/onehot)

Device (8 NeuronCores, SPMD, no collectives; all O(N^2) work):
  * row-shard: each core owns 8 row-tiles of 128 rows (64 tiles cover
    rows 0..8191); the last 7 rows' column sweep (row-tile 64) is
    split column-wise across all 8 cores as class-pure 512-chunks
  * per row-tile: G = rows @ feats^T via PE (bf16 operands, f32 PSUM,
    K=256, 512-col chunks grouped into 1536-col PSUM supertiles)
  * ACT computes exp(A*sim + BIAS) from PSUM into bf16 scratch; DVE
    reduces the class sub-ranges (columns are label-sorted so class
    segments are contiguous and identical on all cores -> the graph
    stays SPMD-uniform)
  * neg_i = total - own-class (selected via shipped onehot); the
    global max subtraction is replaced by the constant M0=12.5 (the
    max only enters through ~1e-8-scale eps terms, verified offline)
  * per-core output: 128-partition partial sums of thresholded loss
    + 3 column-chunk exp sums for the shared row-tile 64
Host: combine 8x[128,8] partials -> scalar.
"""

import ml_dtypes
import numpy as np

import concourse.bass as bass
import concourse.tile as tile
from concourse import mybir
from concourse.bass_utils import run_bass_kernel_spmd

# ---- problem constants (hardcoded per spec) ----
TEMP = 0.08
EPS = 1e-8
GAMMA = 0.99
BETA = 0.5 * (1.0 - GAMMA)
B, D, C = 8192, 256, 7
N = B + C                      # 8199 rows/cols of the score matrix
NCORES = 8
NT = 8                         # full row-tiles per core (8*8*128 = 8192)
ROWS_PER_CORE = NT * 128       # 1024
NPAD = 8704                    # columns padded to 17*512
NF = NPAD + 16                 # + 7 Y columns + zero cols (16-aligned)
SUPER = 1024                   # psum supertile width (2 banks)
T8W = 1536                     # per-core share of row-tile 64's columns
M0 = 12.5                      # constant stand-in for the global max
A_SCALE = 0.5 / float(np.float32(TEMP))
BIAS = (0.5 + EPS) / float(np.float32(TEMP)) - M0

F32 = mybir.dt.float32
BF16 = mybir.dt.bfloat16
FP8 = mybir.dt.float8e4
FP8NP = mybir.dt.np(mybir.dt.float8e4)
ALU = mybir.AluOpType
ACTF = mybir.ActivationFunctionType


def _split_multi_waits(nc):
    """This container's walrus accepts only ONE sync wait per instruction;
    split extra waits into standalone single-wait EventSemaphore insts."""
    n_new = 0
    for func in nc.m.functions:
        for blk in func.blocks:
            new_insts = []
            for inst in blk.instructions:
                si = getattr(inst, "sync_info", None)
                waits = list(si.on_wait) if si and si.on_wait else []
                if len(waits) > 1:
                    for i, w in enumerate(waits[:-1]):
                        n_new += 1
                        ev = mybir.InstEventSemaphore(
                            name=f"{inst.name}-wsplit{i}",
                            engine=inst.engine,
                            ins=[],
                            outs=[],
                            sync_info=mybir.SyncInfo(on_wait=[w], on_update=[]),
                            bass_nofuse=True,
                        )
                        new_insts.append(ev)
                    si.on_wait = [waits[-1]]
                new_insts.append(inst)
            blk.instructions = new_insts
    return n_new


def _host_prep(features, labels, prototypes, momentums):
    # (subranges computed below before per-core metadata uses it)
    features = np.asarray(features, dtype=np.float32)
    labels = np.asarray(labels).astype(np.int64)
    prototypes = np.asarray(prototypes, dtype=np.float32)
    momentums = np.asarray(momentums, dtype=np.float32)

    # ---- prototype update: closed form of the sequential EMA scan ----
    counts_feat = np.bincount(labels, minlength=C)
    rank = np.zeros(B, dtype=np.int64)
    seen = np.zeros(C, dtype=np.int64)
    for i, l in enumerate(labels):
        rank[i] = seen[l]
        seen[l] += 1
    w = BETA * (GAMMA ** (counts_feat[labels] - 1 - rank).astype(np.float64))
    S = np.zeros((C, B))
    S[labels, np.arange(B)] = w
    m_final = S @ features.astype(np.float64)
    wsum = np.bincount(labels, weights=w, minlength=C)
    m_final -= wsum[:, None] * prototypes.astype(np.float64)
    m_final += (GAMMA ** counts_feat.astype(np.float64))[:, None] * momentums.astype(
        np.float64
    )
    target = prototypes.astype(np.float64) + m_final
    q, _ = np.linalg.qr(target.T.astype(np.float32))
    new_protos = q.T.astype(np.float32)

    # ---- normalized, label-sorted gram operands ----
    feats = np.concatenate([features, new_protos], 0)
    labs = np.concatenate([labels, np.arange(C, dtype=np.int64)])
    nrm = np.linalg.norm(feats.astype(np.float64), axis=-1)
    fhat = feats.astype(np.float64) / nrm[:, None]
    perm = np.argsort(labs, kind="stable")
    fs = fhat[perm]
    ls = labs[perm]
    counts_all = np.bincount(ls, minlength=C)          # includes protos
    bounds = np.concatenate([[0], np.cumsum(counts_all)])  # class col ranges

    fs32 = fs.astype(np.float32)
    Y = np.zeros((D, 8), dtype=np.float64)
    for c in range(C):
        Y[:, c] = fs[bounds[c]:bounds[c + 1]].sum(0)

    ftpad = np.zeros((NF, D), dtype=np.float32)
    ftpad[:N] = fs32
    ftpad[NPAD:NPAD + 8] = Y.T.astype(np.float32)
    ft = np.ascontiguousarray(ftpad.T.reshape(2, 128, NF)).astype(
        ml_dtypes.bfloat16
    )  # [k-half, partition, col]

    # ---- ACT sub-ranges: class segments x supertile edges (global) ----
    super_edges = list(range(0, NPAD, SUPER)) + [N]
    edges = sorted(set([int(b) for b in bounds] + super_edges))
    edges = [e for e in edges if e <= N]
    subranges = []  # (super_idx, off_in_super, length, class_id)
    for a, b in zip(edges[:-1], edges[1:]):
        if a >= N:
            break
        cls = int(np.searchsorted(bounds, a, side="right") - 1)
        sup = a // SUPER
        subranges.append((sup, a - sup * SUPER, b - a, cls))
    slot_ranges = []  # per-class slot ranges (contiguous in list order)
    for c in range(C):
        idxs = [i for i, sr in enumerate(subranges) if sr[3] == c]
        slot_ranges.append((min(idxs), max(idxs) + 1))
    n_slots = len(subranges)

    # ---- per-row constants ----
    cnt = counts_all[ls] - 1
    selfsim = (fs32.astype(np.float64) ** 2).sum(1)
    inv = 1.0 / (cnt.astype(np.float64) + EPS)
    alpha_all = A_SCALE * inv
    beta_all = (-A_SCALE * selfsim + BIAS * cnt) * inv

    # ---- shared row-tile 64 (last 7 rows), column-split across cores ----
    t8rows = np.ascontiguousarray(
        ftpad[B:B + 128].T.reshape(2, 128, 128)
    ).astype(ml_dtypes.bfloat16)
    chunk_cols = []  # class-pure 512-col chunks (global col indices)
    chunk_cls = []
    for c in range(C):
        cols = np.arange(bounds[c], bounds[c + 1])
        for o in range(0, len(cols), 512):
            chunk_cols.append(cols[o:o + 512])
            chunk_cls.append(c)
    n_cpc = T8W // 512  # chunks per core
    while len(chunk_cols) < NCORES * n_cpc:
        chunk_cols.append(np.zeros(0, dtype=np.int64))
        chunk_cls.append(-1)
    t8meta = []  # (class, n_pad) per chunk for the host-side combine
    ft_np = np.asarray(ft)
    t8cols_per_core = []
    for core in range(NCORES):
        arr = np.zeros((2, 128, T8W), dtype=ml_dtypes.bfloat16)
        for j in range(n_cpc):
            ci = core * n_cpc + j
            cols = chunk_cols[ci]
            arr[:, :, j * 512:j * 512 + len(cols)] = ft_np[:, :, cols]
            t8meta.append((chunk_cls[ci], 512 - len(cols)))
        t8cols_per_core.append(arr)

    per_core = []
    for core in range(NCORES):
        base = core * ROWS_PER_CORE
        rows_kt = np.ascontiguousarray(
            ftpad[base:base + ROWS_PER_CORE].T.reshape(2, 128, ROWS_PER_CORE)
        ).astype(ml_dtypes.bfloat16)

        onehot = np.zeros((NT, 128, 8), dtype=np.float32)
        rowmeta = np.zeros((128, 3 * NT), dtype=np.float32)  # alpha|beta|valid
        for t in range(NT):
            for p in range(128):
                g = base + t * 128 + p
                onehot[t, p, ls[g]] = 1.0
                rowmeta[p, t] = alpha_all[g]
                rowmeta[p, NT + t] = beta_all[g]
                rowmeta[p, 2 * NT + t] = 1.0
        per_core.append(
            {
                "ft": ft,
                "rows": rows_kt,
                "onehot": onehot,
                "rowmeta": rowmeta,
                "t8rows": t8rows,
                "t8cols": t8cols_per_core[core],
            }
        )

    host = {
        "ls": ls, "bounds": bounds, "counts_all": counts_all, "fs": fs,
        "Y": Y, "t8meta": t8meta, "selfsim": selfsim, "cnt": cnt,
    }
    return per_core, subranges, slot_ranges, n_slots, host


def _build_graph(subranges, slot_ranges, n_slots, bounds):
    nc = bass.Bass()
    ft_d = nc.declare_dram_parameter("ft", [2, 128, NF], BF16, isOutput=False)
    rows_d = nc.declare_dram_parameter(
        "rows", [2, 128, ROWS_PER_CORE], BF16, isOutput=False
    )
    oh_d = nc.declare_dram_parameter("onehot", [NT, 128, 8], F32, isOutput=False)
    meta_d = nc.declare_dram_parameter("rowmeta", [128, 3 * NT], F32, isOutput=False)
    t8r_d = nc.declare_dram_parameter("t8rows", [2, 128, 128], BF16, isOutput=False)
    t8c_d = nc.declare_dram_parameter("t8cols", [2, 128, T8W], BF16, isOutput=False)
    out_d = nc.declare_dram_parameter("out", [128, 8], F32, isOutput=True)

    n_super = (NPAD + SUPER - 1) // SUPER  # 6 (last covers Y cols too)
    super_chunks = []  # per supertile: (ft_off, width, psum_off)
    for s in range(n_super):
        lo = s * SUPER
        hi = min(lo + SUPER, NPAD)
        chunks = [(o, 512, o - lo) for o in range(lo, hi, 512)]
        if s == n_super - 1:
            chunks.append((NPAD, 8, hi - lo))  # Y columns
        super_chunks.append(chunks)
    y_psum_off = NPAD - (n_super - 1) * SUPER  # offset of Y cols in last super

    with tile.TileContext(nc) as tc:
        with (
            tc.tile_pool(name="persist", bufs=1) as persist,
            tc.tile_pool(name="ps", bufs=4, space="PSUM") as psA,
            tc.tile_pool(name="scr", bufs=2) as scrp,
            tc.tile_pool(name="scr8", bufs=2) as scr8p,
            tc.tile_pool(name="slots", bufs=2) as slotp,
            tc.tile_pool(name="small", bufs=4) as small,
        ):
            # --- resident inputs ---
            rows_sb = []
            for k in range(2):
                t_ = persist.tile([128, ROWS_PER_CORE], BF16, tag=f"rows{k}")
                nc.sync.dma_start(out=t_[:], in_=rows_d[k])
                rows_sb.append(t_)
            t8r_sb = []
            for k in range(2):
                t_ = persist.tile([128, 128], BF16, tag=f"t8r{k}")
                nc.sync.dma_start(out=t_[:], in_=t8r_d[k])
                t8r_sb.append(t_)
            t8c_sb = []
            for k in range(2):
                t_ = persist.tile([128, T8W], BF16, tag=f"t8c{k}")
                nc.sync.dma_start(out=t_[:], in_=t8c_d[k])
                t8c_sb.append(t_)
            meta_sb = persist.tile([128, 3 * NT], F32, tag="meta")
            nc.sync.dma_start(out=meta_sb[:], in_=meta_d[:])
            oh_sb = persist.tile([128, NT, 8], F32, tag="oh")
            for t in range(NT):
                nc.sync.dma_start(out=oh_sb[:, t, :], in_=oh_d[t])
            ft_sb = []
            for k in range(2):
                t_ = persist.tile([128, NF], BF16, tag=f"ft{k}")
                ft_sb.append(t_)
            for s in range(n_super):
                lo = s * SUPER
                hi = NF if s == n_super - 1 else lo + SUPER
                for k in range(2):
                    nc.sync.dma_start(out=ft_sb[k][:, lo:hi], in_=ft_d[k, :, lo:hi])

            possel9 = persist.tile([128, NT], F32, tag="possel")
            negsum9 = persist.tile([128, NT], F32, tag="negsum")
            bias_exp = persist.tile([128, 1], F32, tag="bias_exp")
            nc.vector.memset(bias_exp[:], float(BIAS))
            bias_ln = persist.tile([128, 1], F32, tag="bias_ln")
            nc.vector.memset(bias_ln[:], float(EPS))
            out_t = persist.tile([128, 8], F32, tag="out")

            # --- main loop over row-tiles ---
            for t in range(NT):
                scr = scrp.tile([128, NPAD], F32, tag="scr")
                clsum = small.tile([128, 8], F32, tag="csum")
                for s in range(n_super):
                    ps = psA.tile([128, SUPER], F32, tag="ps")
                    reps = [0, 0, 1] if t == 0 else [0, 1]
                    for ri, k in enumerate(reps):
                        last = ri == len(reps) - 1
                        for (off, w, poff) in super_chunks[s]:
                            nc.tensor.matmul(
                                ps[:, poff:poff + w],
                                lhsT=rows_sb[k][:, t * 128:(t + 1) * 128],
                                rhs=ft_sb[k][:, off:off + w],
                                start=(k == 0),
                                stop=last,
                            )
                    lo = s * SUPER
                    hi_valid = min(lo + SUPER, N)
                    if hi_valid > lo:
                        nc.scalar.activation(
                            scr[:, lo:hi_valid],
                            ps[:, 0:hi_valid - lo],
                            ACTF.Exp,
                            bias=bias_exp[:],
                            scale=float(A_SCALE),
                        )
                    if s == n_super - 1:
                        scr7 = small.tile([128, 7], F32, tag="scr7")
                        nc.vector.tensor_tensor(
                            out=scr7[:],
                            in0=ps[:, y_psum_off:y_psum_off + 7],
                            in1=oh_sb[:, t, 0:7],
                            op=ALU.mult,
                        )
                        nc.vector.reduce_sum(
                            possel9[:, t:t + 1], scr7[:], mybir.AxisListType.X
                        )
                # per-class segment sums straight off the big scratch
                for c in range(C):
                    nc.vector.reduce_sum(
                        clsum[:, c:c + 1],
                        scr[:, int(bounds[c]):int(bounds[c + 1])],
                        mybir.AxisListType.X,
                    )
                stot = small.tile([128, 1], F32, tag="stot")
                nc.vector.reduce_sum(stot[:], clsum[:, 0:7], mybir.AxisListType.X)
                scr7b = small.tile([128, 7], F32, tag="scr7b")
                sown = small.tile([128, 1], F32, tag="sown")
                nc.vector.tensor_tensor(
                    out=scr7b[:], in0=clsum[:, 0:7], in1=oh_sb[:, t, 0:7],
                    op=ALU.mult,
                )
                nc.vector.reduce_sum(sown[:], scr7b[:], mybir.AxisListType.X)
                nc.vector.tensor_tensor(
                    out=negsum9[:, t:t + 1], in0=stot[:], in1=sown[:],
                    op=ALU.subtract,
                )

            # --- shared row-tile 64: this core's column slice ---
            ps8 = psA.tile([128, SUPER], F32, tag="ps")
            ps8b = psA.tile([128, SUPER], F32, tag="ps")
            for k in range(2):
                for j in range(T8W // 512):
                    tgt = ps8 if j < 2 else ps8b
                    nc.tensor.matmul(
                        tgt[:, (j % 2) * 512:(j % 2 + 1) * 512],
                        lhsT=t8r_sb[k][:],
                        rhs=t8c_sb[k][:, j * 512:(j + 1) * 512],
                        start=(k == 0),
                        stop=(k == 1),
                    )
            scr8 = scr8p.tile([128, SUPER], F32, tag="scr8")
            scr8b = scr8p.tile([128, SUPER], F32, tag="scr8")
            nc.scalar.activation(
                scr8[:, 0:1024], ps8[:, 0:1024], ACTF.Exp,
                bias=bias_exp[:], scale=float(A_SCALE),
            )
            nc.scalar.activation(
                scr8b[:, 0:512], ps8b[:, 0:512], ACTF.Exp,
                bias=bias_exp[:], scale=float(A_SCALE),
            )
            for j in range(T8W // 512):
                stile = scr8 if j < 2 else scr8b
                nc.vector.reduce_sum(
                    out_t[:, 2 + j:3 + j], stile[:, (j % 2) * 512:(j % 2 + 1) * 512],
                    mybir.AxisListType.X,
                )

            # --- epilogue: loss, threshold, partial sums ---
            alpha9 = meta_sb[:, 0:NT]
            beta9 = meta_sb[:, NT:2 * NT]
            valid9 = meta_sb[:, 2 * NT:3 * NT]
            pos9 = persist.tile([128, NT], F32, tag="pos9")
            nc.vector.tensor_tensor(
                out=pos9[:], in0=possel9[:], in1=alpha9, op=ALU.mult
            )
            nc.vector.tensor_tensor(out=pos9[:], in0=pos9[:], in1=beta9, op=ALU.add)
            neg9 = persist.tile([128, NT], F32, tag="neg9")
            nc.scalar.activation(
                neg9[:], negsum9[:], ACTF.Ln, bias=bias_ln[:], scale=1.0
            )
            loss9 = persist.tile([128, NT], F32, tag="loss9")
            nc.vector.tensor_tensor(
                out=loss9[:], in0=neg9[:], in1=pos9[:], op=ALU.subtract
            )
            gt9 = persist.tile([128, NT], F32, tag="gt9")
            nc.vector.tensor_scalar(
                out=gt9[:], in0=loss9[:], scalar1=0.0, scalar2=None, op0=ALU.is_gt
            )
            nc.vector.tensor_tensor(out=gt9[:], in0=gt9[:], in1=valid9, op=ALU.mult)
            contrib9 = persist.tile([128, NT], F32, tag="contrib9")
            nc.vector.tensor_tensor(
                out=contrib9[:], in0=loss9[:], in1=gt9[:], op=ALU.mult
            )
            nc.vector.reduce_sum(out_t[:, 0:1], contrib9[:], mybir.AxisListType.X)
            nc.vector.reduce_sum(out_t[:, 1:2], gt9[:], mybir.AxisListType.X)
            nc.sync.dma_start(out=out_d[:], in_=out_t[:])
    return nc


def _combine(results, host):
    """Host-side unshard: merge per-core partials + finish row-tile 64."""
    ls = host["ls"]
    fs, Y = host["fs"], host["Y"]
    loss_sum = 0.0
    cnt_sum = 0.0
    for r in results:
        o = np.asarray(r["out"], dtype=np.float64)
        loss_sum += o[:, 0].sum()
        cnt_sum += o[:, 1].sum()

    # row-tile 64: rows 8192..8198 — class sums from per-core chunk sums
    pad_exp = float(
        ml_dtypes.bfloat16(np.exp(np.float32(BIAS)))
    )  # a zero pad column's exp as the device computes it
    n7 = N - B  # 7
    n_cpc = T8W // 512
    classsum = np.zeros((n7, C), dtype=np.float64)
    for core in range(NCORES):
        o = np.asarray(results[core]["out"], dtype=np.float64)
        for j in range(n_cpc):
            cls, n_pad = host["t8meta"][core * n_cpc + j]
            if cls < 0:
                continue
            classsum[:, cls] += o[:n7, 2 + j] - n_pad * pad_exp
    stot = classsum.sum(1)
    rows_ls = ls[B:N]
    sown = classsum[np.arange(n7), rows_ls]
    neg = np.log(stot - sown + EPS)
    pos_sel = np.einsum("id,di->i", fs[B:N], Y[:, rows_ls])
    selfsim = host["selfsim"][B:N]
    cnt = host["cnt"][B:N]
    pos = (A_SCALE * (pos_sel - selfsim) + BIAS * cnt) / (cnt + EPS)
    loss64 = -pos + neg
    m = loss64 > 0
    loss_sum += loss64[m].sum()
    cnt_sum += m.sum()

    val = loss_sum / max(cnt_sum, 1.0) if cnt_sum > 0 else 0.0
    return np.float32(val)


def _run(features, labels, prototypes, momentums, trace=False, trace_kwargs=None):
    per_core, subranges, slot_ranges, n_slots, host = _host_prep(
        features, labels, prototypes, momentums
    )
    nc = _build_graph(subranges, slot_ranges, n_slots, host["bounds"])
    _split_multi_waits(nc)
    in_maps = [per_core[i] for i in range(NCORES)]
    kw = {}
    if trace:
        kw = dict(trace=True, trace_cores=list(range(NCORES)))
        if trace_kwargs:
            kw["trace_kwargs"] = trace_kwargs
    res = run_bass_kernel_spmd(nc, in_maps, core_ids=list(range(NCORES)), **kw)
    return _combine(res.results, host), res


def kernel(features, labels, prototypes, momentums):
    val, _ = _run(features, labels, prototypes, momentums)
    return np.array(val, dtype=np.float32)


# revision 18
# speedup vs baseline: 1.2176x; 1.1395x over previous
"""Trainium2 Bass kernel for AdaptivePrototypeContrastiveLoss.

Strategy
--------
Host (cheap, O(N*D) bookkeeping):
  * closed-form momentum EMA + LAPACK QR -> new prototypes  [7,256]
  * row-normalize feats, stable-sort rows by label, append 7 per-class
    sum columns (Y) so the tiny "positive" matmul rides the main sweep
  * precompute per-row constants (alpha/beta/valid/onehot)

Device (8 NeuronCores, SPMD, no collectives; all O(N^2) work):
  * row-shard: each core owns 8 row-tiles of 128 rows (64 tiles cover
    rows 0..8191); the last 7 rows' column sweep (row-tile 64) is
    split column-wise across all 8 cores as class-pure 512-chunks
  * per row-tile: G = rows @ feats^T via PE (bf16 operands, f32 PSUM,
    K=256, 512-col chunks grouped into 1536-col PSUM supertiles)
  * ACT computes exp(A*sim + BIAS) from PSUM into bf16 scratch; DVE
    reduces the class sub-ranges (columns are label-sorted so class
    segments are contiguous and identical on all cores -> the graph
    stays SPMD-uniform)
  * neg_i = total - own-class (selected via shipped onehot); the
    global max subtraction is replaced by the constant M0=12.5 (the
    max only enters through ~1e-8-scale eps terms, verified offline)
  * per-core output: 128-partition partial sums of thresholded loss
    + 3 column-chunk exp sums for the shared row-tile 64
Host: combine 8x[128,8] partials -> scalar.
"""

import ml_dtypes
import numpy as np

import concourse.bass as bass
import concourse.tile as tile
from concourse import mybir
from concourse.bass_utils import run_bass_kernel_spmd

# ---- problem constants (hardcoded per spec) ----
TEMP = 0.08
EPS = 1e-8
GAMMA = 0.99
BETA = 0.5 * (1.0 - GAMMA)
B, D, C = 8192, 256, 7
N = B + C                      # 8199 rows/cols of the score matrix
NCORES = 8
NT = 8                         # full row-tiles per core (8*8*128 = 8192)
ROWS_PER_CORE = NT * 128       # 1024
NPAD = 8704                    # columns padded to 17*512
NF = NPAD + 16                 # + 7 Y columns + zero cols (16-aligned)
SUPER = 1024                   # psum supertile width (2 banks)
T8W = 1536                     # per-core share of row-tile 64's columns
M0 = 12.5                      # constant stand-in for the global max
A_SCALE = 0.5 / float(np.float32(TEMP))
BIAS = (0.5 + EPS) / float(np.float32(TEMP)) - M0

F32 = mybir.dt.float32
BF16 = mybir.dt.bfloat16
FP8 = mybir.dt.float8e4
FP8NP = mybir.dt.np(mybir.dt.float8e4)
ALU = mybir.AluOpType
ACTF = mybir.ActivationFunctionType


def _split_multi_waits(nc):
    """This container's walrus accepts only ONE sync wait per instruction;
    split extra waits into standalone single-wait EventSemaphore insts."""
    n_new = 0
    for func in nc.m.functions:
        for blk in func.blocks:
            new_insts = []
            for inst in blk.instructions:
                si = getattr(inst, "sync_info", None)
                waits = list(si.on_wait) if si and si.on_wait else []
                if len(waits) > 1:
                    for i, w in enumerate(waits[:-1]):
                        n_new += 1
                        ev = mybir.InstEventSemaphore(
                            name=f"{inst.name}-wsplit{i}",
                            engine=inst.engine,
                            ins=[],
                            outs=[],
                            sync_info=mybir.SyncInfo(on_wait=[w], on_update=[]),
                            bass_nofuse=True,
                        )
                        new_insts.append(ev)
                    si.on_wait = [waits[-1]]
                new_insts.append(inst)
            blk.instructions = new_insts
    return n_new


def _host_prep(features, labels, prototypes, momentums):
    # (subranges computed below before per-core metadata uses it)
    features = np.asarray(features, dtype=np.float32)
    labels = np.asarray(labels).astype(np.int64)
    prototypes = np.asarray(prototypes, dtype=np.float32)
    momentums = np.asarray(momentums, dtype=np.float32)

    # ---- prototype update: closed form of the sequential EMA scan ----
    counts_feat = np.bincount(labels, minlength=C)
    rank = np.zeros(B, dtype=np.int64)
    seen = np.zeros(C, dtype=np.int64)
    for i, l in enumerate(labels):
        rank[i] = seen[l]
        seen[l] += 1
    w = BETA * (GAMMA ** (counts_feat[labels] - 1 - rank).astype(np.float64))
    S = np.zeros((C, B))
    S[labels, np.arange(B)] = w
    m_final = S @ features.astype(np.float64)
    wsum = np.bincount(labels, weights=w, minlength=C)
    m_final -= wsum[:, None] * prototypes.astype(np.float64)
    m_final += (GAMMA ** counts_feat.astype(np.float64))[:, None] * momentums.astype(
        np.float64
    )
    target = prototypes.astype(np.float64) + m_final
    q, _ = np.linalg.qr(target.T.astype(np.float32))
    new_protos = q.T.astype(np.float32)

    # ---- normalized, label-sorted gram operands ----
    feats = np.concatenate([features, new_protos], 0)
    labs = np.concatenate([labels, np.arange(C, dtype=np.int64)])
    nrm = np.linalg.norm(feats.astype(np.float64), axis=-1)
    fhat = feats.astype(np.float64) / nrm[:, None]
    perm = np.argsort(labs, kind="stable")
    fs = fhat[perm]
    ls = labs[perm]
    counts_all = np.bincount(ls, minlength=C)          # includes protos
    bounds = np.concatenate([[0], np.cumsum(counts_all)])  # class col ranges

    fs32 = fs.astype(np.float32)
    Y = np.zeros((D, 8), dtype=np.float64)
    for c in range(C):
        Y[:, c] = fs[bounds[c]:bounds[c + 1]].sum(0)

    ftpad = np.zeros((NF, D), dtype=np.float32)
    ftpad[:N] = fs32
    ftpad[NPAD:NPAD + 8] = Y.T.astype(np.float32)
    ft = np.ascontiguousarray(ftpad.T.reshape(2, 128, NF)).astype(
        ml_dtypes.bfloat16
    )  # [k-half, partition, col]

    # ---- ACT sub-ranges: class segments x supertile edges (global) ----
    super_edges = list(range(0, NPAD, SUPER)) + [N]
    edges = sorted(set([int(b) for b in bounds] + super_edges))
    edges = [e for e in edges if e <= N]
    subranges = []  # (super_idx, off_in_super, length, class_id)
    for a, b in zip(edges[:-1], edges[1:]):
        if a >= N:
            break
        cls = int(np.searchsorted(bounds, a, side="right") - 1)
        sup = a // SUPER
        subranges.append((sup, a - sup * SUPER, b - a, cls))
    slot_ranges = []  # per-class slot ranges (contiguous in list order)
    for c in range(C):
        idxs = [i for i, sr in enumerate(subranges) if sr[3] == c]
        slot_ranges.append((min(idxs), max(idxs) + 1))
    n_slots = len(subranges)

    # ---- per-row constants ----
    cnt = counts_all[ls] - 1
    selfsim = (fs32.astype(np.float64) ** 2).sum(1)
    inv = 1.0 / (cnt.astype(np.float64) + EPS)
    alpha_all = A_SCALE * inv
    beta_all = (-A_SCALE * selfsim + BIAS * cnt) * inv

    # ---- shared row-tile 64 (last 7 rows), column-split across cores ----
    t8rows = np.ascontiguousarray(
        ftpad[B:B + 128].T.reshape(2, 128, 128)
    ).astype(ml_dtypes.bfloat16)
    chunk_cols = []  # class-pure 512-col chunks (global col indices)
    chunk_cls = []
    for c in range(C):
        cols = np.arange(bounds[c], bounds[c + 1])
        for o in range(0, len(cols), 512):
            chunk_cols.append(cols[o:o + 512])
            chunk_cls.append(c)
    n_cpc = T8W // 512  # chunks per core
    while len(chunk_cols) < NCORES * n_cpc:
        chunk_cols.append(np.zeros(0, dtype=np.int64))
        chunk_cls.append(-1)
    t8meta = []  # (class, n_pad) per chunk for the host-side combine
    ft_np = np.asarray(ft)
    t8cols_per_core = []
    for core in range(NCORES):
        arr = np.zeros((2, 128, T8W), dtype=ml_dtypes.bfloat16)
        for j in range(n_cpc):
            ci = core * n_cpc + j
            cols = chunk_cols[ci]
            arr[:, :, j * 512:j * 512 + len(cols)] = ft_np[:, :, cols]
            t8meta.append((chunk_cls[ci], 512 - len(cols)))
        t8cols_per_core.append(arr)

    per_core = []
    for core in range(NCORES):
        base = core * ROWS_PER_CORE
        rows_kt = np.ascontiguousarray(
            ftpad[base:base + ROWS_PER_CORE].T.reshape(2, 128, ROWS_PER_CORE)
        ).astype(ml_dtypes.bfloat16)

        onehot = np.zeros((NT, 128, 24), dtype=np.float32)
        rowmeta = np.zeros((128, 3 * NT), dtype=np.float32)  # alpha|beta|valid
        slot_cls = [sr[3] for sr in subranges]
        for t in range(NT):
            for p in range(128):
                g = base + t * 128 + p
                onehot[t, p, ls[g]] = 1.0
                for si, sc in enumerate(slot_cls):
                    if sc == ls[g]:
                        onehot[t, p, 8 + si] = 1.0
                rowmeta[p, t] = alpha_all[g]
                rowmeta[p, NT + t] = beta_all[g]
                rowmeta[p, 2 * NT + t] = 1.0
        per_core.append(
            {
                "ft": ft,
                "rows": rows_kt,
                "onehot": onehot,
                "rowmeta": rowmeta,
                "t8rows": t8rows,
                "t8cols": t8cols_per_core[core],
            }
        )

    host = {
        "ls": ls, "bounds": bounds, "counts_all": counts_all, "fs": fs,
        "Y": Y, "t8meta": t8meta, "selfsim": selfsim, "cnt": cnt,
    }
    return per_core, subranges, slot_ranges, n_slots, host


def _build_graph(subranges, slot_ranges, n_slots, bounds):
    nc = bass.Bass()
    ft_d = nc.declare_dram_parameter("ft", [2, 128, NF], BF16, isOutput=False)
    rows_d = nc.declare_dram_parameter(
        "rows", [2, 128, ROWS_PER_CORE], BF16, isOutput=False
    )
    oh_d = nc.declare_dram_parameter("onehot", [NT, 128, 24], F32, isOutput=False)
    meta_d = nc.declare_dram_parameter("rowmeta", [128, 3 * NT], F32, isOutput=False)
    t8r_d = nc.declare_dram_parameter("t8rows", [2, 128, 128], BF16, isOutput=False)
    t8c_d = nc.declare_dram_parameter("t8cols", [2, 128, T8W], BF16, isOutput=False)
    out_d = nc.declare_dram_parameter("out", [128, 8], F32, isOutput=True)

    n_super = (NPAD + SUPER - 1) // SUPER  # 6 (last covers Y cols too)
    super_chunks = []  # per supertile: (ft_off, width, psum_off)
    for s in range(n_super):
        lo = s * SUPER
        hi = min(lo + SUPER, NPAD)
        chunks = [(o, 512, o - lo) for o in range(lo, hi, 512)]
        if s == n_super - 1:
            chunks.append((NPAD, 8, hi - lo))  # Y columns
        super_chunks.append(chunks)
    y_psum_off = NPAD - (n_super - 1) * SUPER  # offset of Y cols in last super

    with tile.TileContext(nc) as tc:
        with (
            tc.tile_pool(name="persist", bufs=1) as persist,
            tc.tile_pool(name="ps", bufs=4, space="PSUM") as psA,
            tc.tile_pool(name="scr", bufs=3) as scrp,
            tc.tile_pool(name="scr8", bufs=2) as scr8p,
            tc.tile_pool(name="slots", bufs=2) as slotp,
            tc.tile_pool(name="small", bufs=4) as small,
        ):
            # --- resident inputs ---
            rows_sb = []
            for k in range(2):
                t_ = persist.tile([128, ROWS_PER_CORE], BF16, tag=f"rows{k}")
                nc.sync.dma_start(out=t_[:], in_=rows_d[k])
                rows_sb.append(t_)
            t8r_sb = []
            for k in range(2):
                t_ = persist.tile([128, 128], BF16, tag=f"t8r{k}")
                nc.sync.dma_start(out=t_[:], in_=t8r_d[k])
                t8r_sb.append(t_)
            t8c_sb = []
            for k in range(2):
                t_ = persist.tile([128, T8W], BF16, tag=f"t8c{k}")
                nc.sync.dma_start(out=t_[:], in_=t8c_d[k])
                t8c_sb.append(t_)
            meta_sb = persist.tile([128, 3 * NT], F32, tag="meta")
            nc.sync.dma_start(out=meta_sb[:], in_=meta_d[:])
            oh_sb = persist.tile([128, NT, 24], F32, tag="oh")
            for t in range(NT):
                nc.sync.dma_start(out=oh_sb[:, t, :], in_=oh_d[t])
            ft_sb = []
            for k in range(2):
                t_ = persist.tile([128, NF], BF16, tag=f"ft{k}")
                ft_sb.append(t_)
            for s in range(n_super):
                lo = s * SUPER
                hi = NF if s == n_super - 1 else lo + SUPER
                for k in range(2):
                    nc.sync.dma_start(out=ft_sb[k][:, lo:hi], in_=ft_d[k, :, lo:hi])

            possel9 = persist.tile([128, NT], F32, tag="possel")
            negsum9 = persist.tile([128, NT], F32, tag="negsum")
            bias_exp = persist.tile([128, 1], F32, tag="bias_exp")
            nc.vector.memset(bias_exp[:], float(BIAS))
            bias_ln = persist.tile([128, 1], F32, tag="bias_ln")
            nc.vector.memset(bias_ln[:], float(EPS))
            out_t = persist.tile([128, 8], F32, tag="out")

            # --- main loop over row-tiles ---
            for t in range(NT):
                slots_t = slotp.tile([128, max(n_slots, 8)], F32, tag="slots")
                for s in range(n_super):
                    ps = psA.tile([128, SUPER], F32, tag="ps")
                    reps = [0, 0, 1] if t == 0 else [0, 1]
                    for ri, k in enumerate(reps):
                        last = ri == len(reps) - 1
                        for (off, w, poff) in super_chunks[s]:
                            nc.tensor.matmul(
                                ps[:, poff:poff + w],
                                lhsT=rows_sb[k][:, t * 128:(t + 1) * 128],
                                rhs=ft_sb[k][:, off:off + w],
                                start=(k == 0),
                                stop=last,
                            )
                    scr = scrp.tile([128, SUPER], F32, tag="scr")
                    sub_here = [
                        (si, sr) for si, sr in enumerate(subranges) if sr[0] == s
                    ]
                    if sub_here:
                        lo_off = min(sr[1] for _, sr in sub_here)
                        hi_off = max(sr[1] + sr[2] for _, sr in sub_here)
                        nc.scalar.activation(
                            scr[:, lo_off:hi_off],
                            ps[:, lo_off:hi_off],
                            ACTF.Exp,
                            bias=bias_exp[:],
                            scale=float(A_SCALE),
                        )
                    for si, (sup, off, ln, cls) in sub_here:
                        nc.vector.reduce_sum(
                            slots_t[:, si:si + 1], scr[:, off:off + ln],
                            mybir.AxisListType.X,
                        )
                    if s == n_super - 1:
                        scr7 = small.tile([128, 7], F32, tag="scr7")
                        nc.vector.tensor_tensor(
                            out=scr7[:],
                            in0=ps[:, y_psum_off:y_psum_off + 7],
                            in1=oh_sb[:, t, 0:7],
                            op=ALU.mult,
                        )
                        nc.vector.reduce_sum(
                            possel9[:, t:t + 1], scr7[:], mybir.AxisListType.X
                        )
                # own-class + total sums via slot-level onehot
                stot = small.tile([128, 1], F32, tag="stot")
                nc.vector.reduce_sum(
                    stot[:], slots_t[:, 0:n_slots], mybir.AxisListType.X
                )
                ownsl = small.tile([128, 16], F32, tag="ownsl")
                sown = small.tile([128, 1], F32, tag="sown")
                nc.vector.tensor_tensor(
                    out=ownsl[:, 0:n_slots], in0=slots_t[:, 0:n_slots],
                    in1=oh_sb[:, t, 8:8 + n_slots], op=ALU.mult,
                )
                nc.vector.reduce_sum(
                    sown[:], ownsl[:, 0:n_slots], mybir.AxisListType.X
                )
                nc.vector.tensor_tensor(
                    out=negsum9[:, t:t + 1], in0=stot[:], in1=sown[:],
                    op=ALU.subtract,
                )

            # --- shared row-tile 64: this core's column slice ---
            ps8 = psA.tile([128, SUPER], F32, tag="ps")
            ps8b = psA.tile([128, SUPER], F32, tag="ps")
            for k in range(2):
                for j in range(T8W // 512):
                    tgt = ps8 if j < 2 else ps8b
                    nc.tensor.matmul(
                        tgt[:, (j % 2) * 512:(j % 2 + 1) * 512],
                        lhsT=t8r_sb[k][:],
                        rhs=t8c_sb[k][:, j * 512:(j + 1) * 512],
                        start=(k == 0),
                        stop=(k == 1),
                    )
            scr8 = scr8p.tile([128, SUPER], F32, tag="scr8")
            scr8b = scr8p.tile([128, SUPER], F32, tag="scr8")
            nc.scalar.activation(
                scr8[:, 0:1024], ps8[:, 0:1024], ACTF.Exp,
                bias=bias_exp[:], scale=float(A_SCALE),
            )
            nc.scalar.activation(
                scr8b[:, 0:512], ps8b[:, 0:512], ACTF.Exp,
                bias=bias_exp[:], scale=float(A_SCALE),
            )
            for j in range(T8W // 512):
                stile = scr8 if j < 2 else scr8b
                nc.vector.reduce_sum(
                    out_t[:, 2 + j:3 + j], stile[:, (j % 2) * 512:(j % 2 + 1) * 512],
                    mybir.AxisListType.X,
                )

            # --- epilogue: loss, threshold, partial sums ---
            alpha9 = meta_sb[:, 0:NT]
            beta9 = meta_sb[:, NT:2 * NT]
            valid9 = meta_sb[:, 2 * NT:3 * NT]
            pos9 = persist.tile([128, NT], F32, tag="pos9")
            nc.vector.tensor_tensor(
                out=pos9[:], in0=possel9[:], in1=alpha9, op=ALU.mult
            )
            nc.vector.tensor_tensor(out=pos9[:], in0=pos9[:], in1=beta9, op=ALU.add)
            neg9 = persist.tile([128, NT], F32, tag="neg9")
            nc.scalar.activation(
                neg9[:], negsum9[:], ACTF.Ln, bias=bias_ln[:], scale=1.0
            )
            loss9 = persist.tile([128, NT], F32, tag="loss9")
            nc.vector.tensor_tensor(
                out=loss9[:], in0=neg9[:], in1=pos9[:], op=ALU.subtract
            )
            gt9 = persist.tile([128, NT], F32, tag="gt9")
            nc.vector.tensor_scalar(
                out=gt9[:], in0=loss9[:], scalar1=0.0, scalar2=None, op0=ALU.is_gt
            )
            nc.vector.tensor_tensor(out=gt9[:], in0=gt9[:], in1=valid9, op=ALU.mult)
            contrib9 = persist.tile([128, NT], F32, tag="contrib9")
            nc.vector.tensor_tensor(
                out=contrib9[:], in0=loss9[:], in1=gt9[:], op=ALU.mult
            )
            nc.vector.reduce_sum(out_t[:, 0:1], contrib9[:], mybir.AxisListType.X)
            nc.vector.reduce_sum(out_t[:, 1:2], gt9[:], mybir.AxisListType.X)
            nc.sync.dma_start(out=out_d[:], in_=out_t[:])
    return nc


def _combine(results, host):
    """Host-side unshard: merge per-core partials + finish row-tile 64."""
    ls = host["ls"]
    fs, Y = host["fs"], host["Y"]
    loss_sum = 0.0
    cnt_sum = 0.0
    for r in results:
        o = np.asarray(r["out"], dtype=np.float64)
        loss_sum += o[:, 0].sum()
        cnt_sum += o[:, 1].sum()

    # row-tile 64: rows 8192..8198 — class sums from per-core chunk sums
    pad_exp = float(
        ml_dtypes.bfloat16(np.exp(np.float32(BIAS)))
    )  # a zero pad column's exp as the device computes it
    n7 = N - B  # 7
    n_cpc = T8W // 512
    classsum = np.zeros((n7, C), dtype=np.float64)
    for core in range(NCORES):
        o = np.asarray(results[core]["out"], dtype=np.float64)
        for j in range(n_cpc):
            cls, n_pad = host["t8meta"][core * n_cpc + j]
            if cls < 0:
                continue
            classsum[:, cls] += o[:n7, 2 + j] - n_pad * pad_exp
    stot = classsum.sum(1)
    rows_ls = ls[B:N]
    sown = classsum[np.arange(n7), rows_ls]
    neg = np.log(stot - sown + EPS)
    pos_sel = np.einsum("id,di->i", fs[B:N], Y[:, rows_ls])
    selfsim = host["selfsim"][B:N]
    cnt = host["cnt"][B:N]
    pos = (A_SCALE * (pos_sel - selfsim) + BIAS * cnt) / (cnt + EPS)
    loss64 = -pos + neg
    m = loss64 > 0
    loss_sum += loss64[m].sum()
    cnt_sum += m.sum()

    val = loss_sum / max(cnt_sum, 1.0) if cnt_sum > 0 else 0.0
    return np.float32(val)


def _run(features, labels, prototypes, momentums, trace=False, trace_kwargs=None):
    per_core, subranges, slot_ranges, n_slots, host = _host_prep(
        features, labels, prototypes, momentums
    )
    nc = _build_graph(subranges, slot_ranges, n_slots, host["bounds"])
    _split_multi_waits(nc)
    in_maps = [per_core[i] for i in range(NCORES)]
    kw = {}
    if trace:
        kw = dict(trace=True, trace_cores=list(range(NCORES)))
        if trace_kwargs:
            kw["trace_kwargs"] = trace_kwargs
    res = run_bass_kernel_spmd(nc, in_maps, core_ids=list(range(NCORES)), **kw)
    return _combine(res.results, host), res


def kernel(features, labels, prototypes, momentums):
    val, _ = _run(features, labels, prototypes, momentums)
    return np.array(val, dtype=np.float32)


# revision 19
# speedup vs baseline: 1.2249x; 1.0060x over previous
"""Trainium2 Bass kernel for AdaptivePrototypeContrastiveLoss.

Strategy
--------
Host (cheap, O(N*D) bookkeeping):
  * closed-form momentum EMA + LAPACK QR -> new prototypes  [7,256]
  * row-normalize feats, stable-sort rows by label, append 7 per-class
    sum columns (Y) so the tiny "positive" matmul rides the main sweep
  * precompute per-row constants (alpha/beta/valid/onehot)

Device (8 NeuronCores, SPMD, no collectives; all O(N^2) work):
  * row-shard: each core owns 8 row-tiles of 128 rows (64 tiles cover
    rows 0..8191); the last 7 rows' column sweep (row-tile 64) is
    split column-wise across all 8 cores as class-pure 512-chunks
  * per row-tile: G = rows @ feats^T via PE (bf16 operands, f32 PSUM,
    K=256, 512-col chunks grouped into 1536-col PSUM supertiles)
  * ACT computes exp(A*sim + BIAS) from PSUM into bf16 scratch; DVE
    reduces the class sub-ranges (columns are label-sorted so class
    segments are contiguous and identical on all cores -> the graph
    stays SPMD-uniform)
  * neg_i = total - own-class (selected via shipped onehot); the
    global max subtraction is replaced by the constant M0=12.5 (the
    max only enters through ~1e-8-scale eps terms, verified offline)
  * per-core output: 128-partition partial sums of thresholded loss
    + 3 column-chunk exp sums for the shared row-tile 64
Host: combine 8x[128,8] partials -> scalar.
"""

import ml_dtypes
import numpy as np

import concourse.bass as bass
import concourse.tile as tile
from concourse import mybir
from concourse.bass_utils import run_bass_kernel_spmd

# ---- problem constants (hardcoded per spec) ----
TEMP = 0.08
EPS = 1e-8
GAMMA = 0.99
BETA = 0.5 * (1.0 - GAMMA)
B, D, C = 8192, 256, 7
N = B + C                      # 8199 rows/cols of the score matrix
NCORES = 8
NT = 8                         # full row-tiles per core (8*8*128 = 8192)
ROWS_PER_CORE = NT * 128       # 1024
NPAD = 8704                    # columns padded to 17*512
NF = NPAD + 16                 # + 7 Y columns + zero cols (16-aligned)
SUPER = 1024                   # psum supertile width (2 banks)
T8W = 1536                     # per-core share of row-tile 64's columns
M0 = 12.5                      # constant stand-in for the global max
A_SCALE = 0.5 / float(np.float32(TEMP))
BIAS = (0.5 + EPS) / float(np.float32(TEMP)) - M0

F32 = mybir.dt.float32
BF16 = mybir.dt.bfloat16
FP8 = mybir.dt.float8e4
FP8NP = mybir.dt.np(mybir.dt.float8e4)
ALU = mybir.AluOpType
ACTF = mybir.ActivationFunctionType


def _split_multi_waits(nc):
    """This container's walrus accepts only ONE sync wait per instruction;
    split extra waits into standalone single-wait EventSemaphore insts."""
    n_new = 0
    for func in nc.m.functions:
        for blk in func.blocks:
            new_insts = []
            for inst in blk.instructions:
                si = getattr(inst, "sync_info", None)
                waits = list(si.on_wait) if si and si.on_wait else []
                if len(waits) > 1:
                    for i, w in enumerate(waits[:-1]):
                        n_new += 1
                        ev = mybir.InstEventSemaphore(
                            name=f"{inst.name}-wsplit{i}",
                            engine=inst.engine,
                            ins=[],
                            outs=[],
                            sync_info=mybir.SyncInfo(on_wait=[w], on_update=[]),
                            bass_nofuse=True,
                        )
                        new_insts.append(ev)
                    si.on_wait = [waits[-1]]
                new_insts.append(inst)
            blk.instructions = new_insts
    return n_new


def _host_prep(features, labels, prototypes, momentums):
    # (subranges computed below before per-core metadata uses it)
    features = np.asarray(features, dtype=np.float32)
    labels = np.asarray(labels).astype(np.int64)
    prototypes = np.asarray(prototypes, dtype=np.float32)
    momentums = np.asarray(momentums, dtype=np.float32)

    # ---- prototype update: closed form of the sequential EMA scan ----
    counts_feat = np.bincount(labels, minlength=C)
    rank = np.zeros(B, dtype=np.int64)
    seen = np.zeros(C, dtype=np.int64)
    for i, l in enumerate(labels):
        rank[i] = seen[l]
        seen[l] += 1
    w = BETA * (GAMMA ** (counts_feat[labels] - 1 - rank).astype(np.float64))
    S = np.zeros((C, B))
    S[labels, np.arange(B)] = w
    m_final = S @ features.astype(np.float64)
    wsum = np.bincount(labels, weights=w, minlength=C)
    m_final -= wsum[:, None] * prototypes.astype(np.float64)
    m_final += (GAMMA ** counts_feat.astype(np.float64))[:, None] * momentums.astype(
        np.float64
    )
    target = prototypes.astype(np.float64) + m_final
    q, _ = np.linalg.qr(target.T.astype(np.float32))
    new_protos = q.T.astype(np.float32)

    # ---- normalized, label-sorted gram operands ----
    feats = np.concatenate([features, new_protos], 0)
    labs = np.concatenate([labels, np.arange(C, dtype=np.int64)])
    nrm = np.linalg.norm(feats.astype(np.float64), axis=-1)
    fhat = feats.astype(np.float64) / nrm[:, None]
    perm = np.argsort(labs, kind="stable")
    fs = fhat[perm]
    ls = labs[perm]
    counts_all = np.bincount(ls, minlength=C)          # includes protos
    bounds = np.concatenate([[0], np.cumsum(counts_all)])  # class col ranges

    fs32 = fs.astype(np.float32)
    Y = np.zeros((D, 8), dtype=np.float64)
    for c in range(C):
        Y[:, c] = fs[bounds[c]:bounds[c + 1]].sum(0)

    ftpad = np.zeros((NF, D), dtype=np.float32)
    ftpad[:N] = fs32
    ftpad[NPAD:NPAD + 8] = Y.T.astype(np.float32)
    ft = np.ascontiguousarray(ftpad.T.reshape(2, 128, NF)).astype(
        ml_dtypes.bfloat16
    )  # [k-half, partition, col]

    # ---- ACT sub-ranges: class segments x supertile edges (global) ----
    super_edges = list(range(0, NPAD, SUPER)) + [N]
    edges = sorted(set([int(b) for b in bounds] + super_edges))
    edges = [e for e in edges if e <= N]
    subranges = []  # (super_idx, off_in_super, length, class_id)
    for a, b in zip(edges[:-1], edges[1:]):
        if a >= N:
            break
        cls = int(np.searchsorted(bounds, a, side="right") - 1)
        sup = a // SUPER
        subranges.append((sup, a - sup * SUPER, b - a, cls))
    slot_ranges = []  # per-class slot ranges (contiguous in list order)
    for c in range(C):
        idxs = [i for i, sr in enumerate(subranges) if sr[3] == c]
        slot_ranges.append((min(idxs), max(idxs) + 1))
    n_slots = len(subranges)

    # ---- per-row constants ----
    cnt = counts_all[ls] - 1
    selfsim = (fs32.astype(np.float64) ** 2).sum(1)
    inv = 1.0 / (cnt.astype(np.float64) + EPS)
    alpha_all = A_SCALE * inv
    beta_all = (-A_SCALE * selfsim + BIAS * cnt) * inv

    # ---- shared row-tile 64 (last 7 rows), column-split across cores ----
    t8rows = np.ascontiguousarray(
        ftpad[B:B + 128].T.reshape(2, 128, 128)
    ).astype(ml_dtypes.bfloat16)
    chunk_cols = []  # class-pure 512-col chunks (global col indices)
    chunk_cls = []
    for c in range(C):
        cols = np.arange(bounds[c], bounds[c + 1])
        for o in range(0, len(cols), 512):
            chunk_cols.append(cols[o:o + 512])
            chunk_cls.append(c)
    n_cpc = T8W // 512  # chunks per core
    while len(chunk_cols) < NCORES * n_cpc:
        chunk_cols.append(np.zeros(0, dtype=np.int64))
        chunk_cls.append(-1)
    t8meta = []  # (class, n_pad) per chunk for the host-side combine
    ft_np = np.asarray(ft)
    t8cols_per_core = []
    for core in range(NCORES):
        arr = np.zeros((2, 128, T8W), dtype=ml_dtypes.bfloat16)
        for j in range(n_cpc):
            ci = core * n_cpc + j
            cols = chunk_cols[ci]
            arr[:, :, j * 512:j * 512 + len(cols)] = ft_np[:, :, cols]
            t8meta.append((chunk_cls[ci], 512 - len(cols)))
        t8cols_per_core.append(arr)

    per_core = []
    for core in range(NCORES):
        base = core * ROWS_PER_CORE
        rows_kt = np.ascontiguousarray(
            ftpad[base:base + ROWS_PER_CORE].T.reshape(2, 128, ROWS_PER_CORE)
        ).astype(ml_dtypes.bfloat16)

        onehot = np.zeros((NT, 128, 24), dtype=np.float32)
        rowmeta = np.zeros((128, 3 * NT), dtype=np.float32)  # alpha|beta|valid
        slot_cls = [sr[3] for sr in subranges]
        for t in range(NT):
            for p in range(128):
                g = base + t * 128 + p
                onehot[t, p, ls[g]] = 1.0
                for si, sc in enumerate(slot_cls):
                    if sc == ls[g]:
                        onehot[t, p, 8 + si] = 1.0
                rowmeta[p, t] = alpha_all[g]
                rowmeta[p, NT + t] = beta_all[g]
                rowmeta[p, 2 * NT + t] = 1.0
        per_core.append(
            {
                "ft": ft,
                "rows": rows_kt,
                "onehot": onehot,
                "rowmeta": rowmeta,
                "t8rows": t8rows,
                "t8cols": t8cols_per_core[core],
            }
        )

    host = {
        "ls": ls, "bounds": bounds, "counts_all": counts_all, "fs": fs,
        "Y": Y, "t8meta": t8meta, "selfsim": selfsim, "cnt": cnt,
    }
    return per_core, subranges, slot_ranges, n_slots, host


def _build_graph(subranges, slot_ranges, n_slots, bounds):
    nc = bass.Bass()
    ft_d = nc.declare_dram_parameter("ft", [2, 128, NF], BF16, isOutput=False)
    rows_d = nc.declare_dram_parameter(
        "rows", [2, 128, ROWS_PER_CORE], BF16, isOutput=False
    )
    oh_d = nc.declare_dram_parameter("onehot", [NT, 128, 24], F32, isOutput=False)
    meta_d = nc.declare_dram_parameter("rowmeta", [128, 3 * NT], F32, isOutput=False)
    t8r_d = nc.declare_dram_parameter("t8rows", [2, 128, 128], BF16, isOutput=False)
    t8c_d = nc.declare_dram_parameter("t8cols", [2, 128, T8W], BF16, isOutput=False)
    out_d = nc.declare_dram_parameter("out", [128, 8], F32, isOutput=True)

    n_super = (NPAD + SUPER - 1) // SUPER  # 6 (last covers Y cols too)
    super_chunks = []  # per supertile: (ft_off, width, psum_off)
    for s in range(n_super):
        lo = s * SUPER
        hi = min(lo + SUPER, NPAD)
        chunks = [(o, 512, o - lo) for o in range(lo, hi, 512)]
        if s == n_super - 1:
            chunks.append((NPAD, 8, hi - lo))  # Y columns
        super_chunks.append(chunks)
    y_psum_off = NPAD - (n_super - 1) * SUPER  # offset of Y cols in last super

    with tile.TileContext(nc) as tc:
        with (
            tc.tile_pool(name="persist", bufs=1) as persist,
            tc.tile_pool(name="ps", bufs=4, space="PSUM") as psA,
            tc.tile_pool(name="scr", bufs=3) as scrp,
            tc.tile_pool(name="scr8", bufs=2) as scr8p,
            tc.tile_pool(name="slots", bufs=2) as slotp,
            tc.tile_pool(name="small", bufs=4) as small,
        ):
            # --- resident inputs ---
            rows_sb = []
            for k in range(2):
                t_ = persist.tile([128, ROWS_PER_CORE], BF16, tag=f"rows{k}")
                nc.sync.dma_start(out=t_[:], in_=rows_d[k])
                rows_sb.append(t_)
            t8r_sb = []
            for k in range(2):
                t_ = persist.tile([128, 128], BF16, tag=f"t8r{k}")
                nc.sync.dma_start(out=t_[:], in_=t8r_d[k])
                t8r_sb.append(t_)
            t8c_sb = []
            for k in range(2):
                t_ = persist.tile([128, T8W], BF16, tag=f"t8c{k}")
                nc.sync.dma_start(out=t_[:], in_=t8c_d[k])
                t8c_sb.append(t_)
            meta_sb = persist.tile([128, 3 * NT], F32, tag="meta")
            nc.sync.dma_start(out=meta_sb[:], in_=meta_d[:])
            oh_sb = persist.tile([128, NT, 24], F32, tag="oh")
            for t in range(NT):
                nc.sync.dma_start(out=oh_sb[:, t, :], in_=oh_d[t])
            ft_sb = []
            for k in range(2):
                t_ = persist.tile([128, NF], BF16, tag=f"ft{k}")
                ft_sb.append(t_)
            for s in range(n_super):
                lo = s * SUPER
                hi = NF if s == n_super - 1 else lo + SUPER
                for k in range(2):
                    nc.sync.dma_start(out=ft_sb[k][:, lo:hi], in_=ft_d[k, :, lo:hi])

            possel9 = persist.tile([128, NT], F32, tag="possel")
            negsum9 = persist.tile([128, NT], F32, tag="negsum")
            bias_exp = persist.tile([128, 1], F32, tag="bias_exp")
            nc.vector.memset(bias_exp[:], float(BIAS))
            bias_ln = persist.tile([128, 1], F32, tag="bias_ln")
            nc.vector.memset(bias_ln[:], float(EPS))
            out_t = persist.tile([128, 8], F32, tag="out")

            # --- main loop over row-tiles ---
            for t in range(NT):
                slots_t = slotp.tile([128, max(n_slots, 8)], F32, tag="slots")
                for s in range(n_super):
                    ps = psA.tile([128, SUPER], F32, tag="ps")
                    reps = [0, 1]
                    for ri, k in enumerate(reps):
                        last = ri == len(reps) - 1
                        for (off, w, poff) in super_chunks[s]:
                            nc.tensor.matmul(
                                ps[:, poff:poff + w],
                                lhsT=rows_sb[k][:, t * 128:(t + 1) * 128],
                                rhs=ft_sb[k][:, off:off + w],
                                start=(k == 0),
                                stop=last,
                            )
                    scr = scrp.tile([128, SUPER], F32, tag="scr")
                    sub_here = [
                        (si, sr) for si, sr in enumerate(subranges) if sr[0] == s
                    ]
                    if sub_here:
                        lo_off = min(sr[1] for _, sr in sub_here)
                        hi_off = max(sr[1] + sr[2] for _, sr in sub_here)
                        nc.scalar.activation(
                            scr[:, lo_off:hi_off],
                            ps[:, lo_off:hi_off],
                            ACTF.Exp,
                            bias=bias_exp[:],
                            scale=float(A_SCALE),
                        )
                    for si, (sup, off, ln, cls) in sub_here:
                        nc.vector.reduce_sum(
                            slots_t[:, si:si + 1], scr[:, off:off + ln],
                            mybir.AxisListType.X,
                        )
                    if s == n_super - 1:
                        scr7 = small.tile([128, 7], F32, tag="scr7")
                        nc.vector.tensor_tensor(
                            out=scr7[:],
                            in0=ps[:, y_psum_off:y_psum_off + 7],
                            in1=oh_sb[:, t, 0:7],
                            op=ALU.mult,
                        )
                        nc.vector.reduce_sum(
                            possel9[:, t:t + 1], scr7[:], mybir.AxisListType.X
                        )
                # own-class + total sums via slot-level onehot
                stot = small.tile([128, 1], F32, tag="stot")
                nc.vector.reduce_sum(
                    stot[:], slots_t[:, 0:n_slots], mybir.AxisListType.X
                )
                ownsl = small.tile([128, 16], F32, tag="ownsl")
                sown = small.tile([128, 1], F32, tag="sown")
                nc.vector.tensor_tensor(
                    out=ownsl[:, 0:n_slots], in0=slots_t[:, 0:n_slots],
                    in1=oh_sb[:, t, 8:8 + n_slots], op=ALU.mult,
                )
                nc.vector.reduce_sum(
                    sown[:], ownsl[:, 0:n_slots], mybir.AxisListType.X
                )
                nc.vector.tensor_tensor(
                    out=negsum9[:, t:t + 1], in0=stot[:], in1=sown[:],
                    op=ALU.subtract,
                )

            # --- shared row-tile 64: this core's column slice ---
            ps8 = psA.tile([128, SUPER], F32, tag="ps")
            ps8b = psA.tile([128, SUPER], F32, tag="ps")
            for k in range(2):
                for j in range(T8W // 512):
                    tgt = ps8 if j < 2 else ps8b
                    nc.tensor.matmul(
                        tgt[:, (j % 2) * 512:(j % 2 + 1) * 512],
                        lhsT=t8r_sb[k][:],
                        rhs=t8c_sb[k][:, j * 512:(j + 1) * 512],
                        start=(k == 0),
                        stop=(k == 1),
                    )
            scr8 = scr8p.tile([128, SUPER], F32, tag="scr8")
            scr8b = scr8p.tile([128, SUPER], F32, tag="scr8")
            nc.scalar.activation(
                scr8[:, 0:1024], ps8[:, 0:1024], ACTF.Exp,
                bias=bias_exp[:], scale=float(A_SCALE),
            )
            nc.scalar.activation(
                scr8b[:, 0:512], ps8b[:, 0:512], ACTF.Exp,
                bias=bias_exp[:], scale=float(A_SCALE),
            )
            for j in range(T8W // 512):
                stile = scr8 if j < 2 else scr8b
                nc.vector.reduce_sum(
                    out_t[:, 2 + j:3 + j], stile[:, (j % 2) * 512:(j % 2 + 1) * 512],
                    mybir.AxisListType.X,
                )

            # --- epilogue: loss, threshold, partial sums ---
            alpha9 = meta_sb[:, 0:NT]
            beta9 = meta_sb[:, NT:2 * NT]
            valid9 = meta_sb[:, 2 * NT:3 * NT]
            pos9 = persist.tile([128, NT], F32, tag="pos9")
            nc.vector.tensor_tensor(
                out=pos9[:], in0=possel9[:], in1=alpha9, op=ALU.mult
            )
            nc.vector.tensor_tensor(out=pos9[:], in0=pos9[:], in1=beta9, op=ALU.add)
            neg9 = persist.tile([128, NT], F32, tag="neg9")
            nc.scalar.activation(
                neg9[:], negsum9[:], ACTF.Ln, bias=bias_ln[:], scale=1.0
            )
            loss9 = persist.tile([128, NT], F32, tag="loss9")
            nc.vector.tensor_tensor(
                out=loss9[:], in0=neg9[:], in1=pos9[:], op=ALU.subtract
            )
            gt9 = persist.tile([128, NT], F32, tag="gt9")
            nc.vector.tensor_scalar(
                out=gt9[:], in0=loss9[:], scalar1=0.0, scalar2=None, op0=ALU.is_gt
            )
            nc.vector.tensor_tensor(out=gt9[:], in0=gt9[:], in1=valid9, op=ALU.mult)
            contrib9 = persist.tile([128, NT], F32, tag="contrib9")
            nc.vector.tensor_tensor(
                out=contrib9[:], in0=loss9[:], in1=gt9[:], op=ALU.mult
            )
            nc.vector.reduce_sum(out_t[:, 0:1], contrib9[:], mybir.AxisListType.X)
            nc.vector.reduce_sum(out_t[:, 1:2], gt9[:], mybir.AxisListType.X)
            nc.sync.dma_start(out=out_d[:], in_=out_t[:])
    return nc


def _combine(results, host):
    """Host-side unshard: merge per-core partials + finish row-tile 64."""
    ls = host["ls"]
    fs, Y = host["fs"], host["Y"]
    loss_sum = 0.0
    cnt_sum = 0.0
    for r in results:
        o = np.asarray(r["out"], dtype=np.float64)
        loss_sum += o[:, 0].sum()
        cnt_sum += o[:, 1].sum()

    # row-tile 64: rows 8192..8198 — class sums from per-core chunk sums
    pad_exp = float(
        ml_dtypes.bfloat16(np.exp(np.float32(BIAS)))
    )  # a zero pad column's exp as the device computes it
    n7 = N - B  # 7
    n_cpc = T8W // 512
    classsum = np.zeros((n7, C), dtype=np.float64)
    for core in range(NCORES):
        o = np.asarray(results[core]["out"], dtype=np.float64)
        for j in range(n_cpc):
            cls, n_pad = host["t8meta"][core * n_cpc + j]
            if cls < 0:
                continue
            classsum[:, cls] += o[:n7, 2 + j] - n_pad * pad_exp
    stot = classsum.sum(1)
    rows_ls = ls[B:N]
    sown = classsum[np.arange(n7), rows_ls]
    neg = np.log(stot - sown + EPS)
    pos_sel = np.einsum("id,di->i", fs[B:N], Y[:, rows_ls])
    selfsim = host["selfsim"][B:N]
    cnt = host["cnt"][B:N]
    pos = (A_SCALE * (pos_sel - selfsim) + BIAS * cnt) / (cnt + EPS)
    loss64 = -pos + neg
    m = loss64 > 0
    loss_sum += loss64[m].sum()
    cnt_sum += m.sum()

    val = loss_sum / max(cnt_sum, 1.0) if cnt_sum > 0 else 0.0
    return np.float32(val)


def _run(features, labels, prototypes, momentums, trace=False, trace_kwargs=None):
    per_core, subranges, slot_ranges, n_slots, host = _host_prep(
        features, labels, prototypes, momentums
    )
    nc = _build_graph(subranges, slot_ranges, n_slots, host["bounds"])
    _split_multi_waits(nc)
    in_maps = [per_core[i] for i in range(NCORES)]
    kw = {}
    if trace:
        kw = dict(trace=True, trace_cores=list(range(NCORES)))
        if trace_kwargs:
            kw["trace_kwargs"] = trace_kwargs
    res = run_bass_kernel_spmd(nc, in_maps, core_ids=list(range(NCORES)), **kw)
    return _combine(res.results, host), res


def kernel(features, labels, prototypes, momentums):
    val, _ = _run(features, labels, prototypes, momentums)
    return np.array(val, dtype=np.float32)
